# revision 11
# baseline (speedup 1.0000x reference)
"""Trainium2 Bass kernel for the attention layer:

    f = wf@x+bf; g = wg@x+bg; h = wh@x+bh            (1x1 convs, Ci=32)
    attn = softmax(f^T g, axis=-1)                   (per batch, N=4096)
    out = (wv @ (h @ attn^T) + bv) * gamma + x

Sharding: 8 cores = 4 batches x 2 query-halves (2048 queries each).
Each core receives the full (256, 4096) batch slice with its query half
permuted to the front, so the SPMD program uses fixed offsets.

Per-core dataflow (matmuls fp32r, PSUM fp32 accumulate):
  - warm-up: a dense block of dummy matmuls at t=0 so the PE HAM clock
    gate reaches 8/8 before the real work, plus a dummy exp to pull the
    ACT table load forward.
  - f/g are computed replicated onto 4 partition strips (host-replicated
    wf^T/wg^T with M=128), so the K=32 logits matmuls can be row-packed
    with tile_position: consecutive key chunks run concurrently in PE
    row bands, each writing its own PSUM bank.
  - hT (4096, 32) k-major blocks: lhsT=x k-chunk, rhs=wh^T.
  - per 512-query chunk: 32 k-chunk matmuls logitsT = g^T f (k on
    partitions, row-packed) -> ACT exp PSUM->SBUF (1024 wide) -> 32
    k-chunk accumulation rounds, each round two CONCURRENT column-tiled
    matmuls into one PSUM bank: rows 0-31 accumulate the softmax
    denominator (ones stationary), rows 32-63 accumulate x0 = h@attn^T
    (hT stationary). Reciprocal of row 0, GPSIMD partition-broadcast,
    multiply -> x0a; project with wv*gamma; bias (bv+wv@bh folded on
    host) + residual fused in one scalar_tensor_tensor; DMA out.
"""

import os
import numpy as np

import concourse.bass as bass
import concourse.mybir as mybir
import concourse.tile as tile
from concourse import bacc
from concourse.bass import ts
from concourse.bass_utils import run_bass_kernel_spmd

F32 = mybir.dt.float32
F32R = mybir.dt.float32r
EXP = mybir.ActivationFunctionType.Exp
ADD = mybir.AluOpType.add

B, C, W, H = 4, 256, 64, 64
N = W * H            # 4096 keys/queries per batch
CI = 32              # inner channels
NCORES = 8
NQ = N // 2          # queries per core
QC = 512             # query chunk = one fp32 PSUM bank
NQC = NQ // QC       # 4 query chunks per core
KC = 128             # key chunk = partition dim
NKC = N // KC        # 32 key chunks
GRP = 2              # key chunks per ACT exp group (PSUM banks per tile)
NWARM = 8            # dummy fp32 matmuls to warm the PE clock gate

# Trace knob for test harnesses: set kernel.TRACE = True to profile.
TRACE = False
LAST_EXEC_NS = None

_cached_nc = None


def _mm(nc, out, lhsT, rhs, start, stop, tile_position=None):
    nc.tensor.matmul(out, lhsT=lhsT, rhs=rhs, start=start, stop=stop,
                     tile_position=tile_position)


def _build():
    nc = bacc.Bacc(
        "TRN2", target_bir_lowering=False, debug=False, num_devices=NCORES
    )
    x_d = nc.dram_tensor("x", (C, N), F32R, kind="ExternalInput").ap()
    wfT_d = nc.dram_tensor("wfT", (C, 128), F32R, kind="ExternalInput").ap()
    wgT_d = nc.dram_tensor("wgT", (C, 128), F32R, kind="ExternalInput").ap()
    whT_d = nc.dram_tensor("whT", (C, CI), F32R, kind="ExternalInput").ap()
    wvT_d = nc.dram_tensor("wvT", (CI + 1, C), F32R, kind="ExternalInput").ap()
    bf_d = nc.dram_tensor("bf", (128, 1), F32, kind="ExternalInput").ap()
    bg_d = nc.dram_tensor("bg", (128, 1), F32, kind="ExternalInput").ap()
    out_d = nc.dram_tensor("out", (C, NQ), F32, kind="ExternalOutput").ap()

    xr = x_d.rearrange("(cc p) n -> p cc n", p=128)
    outr = out_d.rearrange("(oc p) n -> p oc n", p=128)

    with tile.TileContext(nc) as tc:
        with (
            tc.tile_pool(name="consts", bufs=1) as consts,
            tc.tile_pool(name="data", bufs=1) as data,
            tc.tile_pool(name="eTp", bufs=4) as eTp,
            tc.tile_pool(name="smallp", bufs=2) as smallp,
            tc.tile_pool(name="outp", bufs=3) as outp,
            tc.tile_pool(name="pl", bufs=2, space="PSUM") as pl,
            tc.tile_pool(name="pp", bufs=2, space="PSUM") as pp,
            tc.tile_pool(name="px0", bufs=2, space="PSUM") as px0,
        ):
            # ---- PE + ACT warm-up (overlaps the input DMAs) ----
            scratch = consts.tile([128, QC], F32)
            nc.vector.memset(scratch, 0.0)
            wps = pp.tile([128, QC], F32, tag="pp")
            for i in range(NWARM):
                nc.tensor.matmul(
                    wps, lhsT=scratch[:, 0:128], rhs=scratch,
                    start=True, stop=True, skip_group_check=True,
                )
            scratch2 = consts.tile([1, 8], F32)
            nc.scalar.activation(
                out=scratch2, in_=scratch[0:1, 0:8], func=EXP
            )

            # ---- constants ----
            wfT_sb = consts.tile([128, 2, 128], F32R)
            nc.sync.dma_start(
                out=wfT_sb, in_=wfT_d.rearrange("(cc p) o -> p cc o", p=128)
            )
            wgT_sb = consts.tile([128, 2, 128], F32R)
            nc.sync.dma_start(
                out=wgT_sb, in_=wgT_d.rearrange("(cc p) o -> p cc o", p=128)
            )
            whT_sb = consts.tile([128, 2, CI], F32R)
            nc.sync.dma_start(
                out=whT_sb, in_=whT_d.rearrange("(cc p) o -> p cc o", p=128)
            )
            wvT_sb = consts.tile([CI + 1, 2, 128], F32R)
            nc.sync.dma_start(
                out=wvT_sb, in_=wvT_d.rearrange("p (oc m) -> p oc m", oc=2)
            )
            bf_sb = consts.tile([128, 1], F32)
            nc.sync.dma_start(out=bf_sb, in_=bf_d)
            bg_sb = consts.tile([128, 1], F32)
            nc.sync.dma_start(out=bg_sb, in_=bg_d)
            ones_sb = consts.tile([128, 1], F32)
            nc.vector.memset(ones_sb, 1.0)
            scratchR = consts.tile([128, QC], F32R)
            nc.vector.tensor_copy(scratchR, scratch)

            # ---- x ----
            x_sb = data.tile([128, 2, N], F32R)
            for s in range(4):
                nc.sync.dma_start(
                    out=x_sb[:, :, ts(s, N // 4)], in_=xr[:, :, ts(s, N // 4)]
                )

            # ---- f, g (replicated on 4 strips), hT ----
            f_sb = data.tile([128, NQ], F32R)
            g_sb = data.tile([128, N], F32R)
            hT_sb = data.tile([128, NKC, CI + 1], F32R)
            nc.vector.tensor_copy(
                hT_sb[:, :, 0:1], ones_sb.to_broadcast([128, NKC, 1])
            )

            def emit_f(j):
                ps = pp.tile([128, QC], F32, tag="pp", name=f"psf{j}")
                for cc in range(2):
                    _mm(nc, ps, wfT_sb[:, cc, :],
                        x_sb[:, cc, ts(j, QC)], cc == 0, cc == 1)
                nc.vector.tensor_scalar_add(
                    f_sb[:, ts(j, QC)], ps, bf_sb
                )

            def emit_g(j):
                ps = pp.tile([128, QC], F32, tag="pp", name=f"psg{j}")
                for cc in range(2):
                    _mm(nc, ps, wgT_sb[:, cc, :],
                        x_sb[:, cc, ts(j, QC)], cc == 0, cc == 1)
                nc.vector.tensor_scalar_add(
                    g_sb[:, ts(j, QC)], ps, bg_sb
                )

            def emit_hT(kc):
                ps = pp.tile([128, QC], F32, tag="pp", name=f"psh{kc}")
                for cc in range(2):
                    _mm(nc, ps[:, 0:CI], x_sb[:, cc, ts(kc, KC)],
                        whT_sb[:, cc, :], cc == 0, cc == 1)
                nc.vector.tensor_copy(hT_sb[:, kc, 1 : CI + 1], ps[:, 0:CI])

            # emit order sets scheduler priority: produce what the main
            # loop's first groups need first, so the q-loop starts early
            # and the rest of this phase fills PE gaps under the
            # ACT-bound steady state.
            emit_f(0)
            for j in range(N // QC):
                emit_g(j)
                for kc in range(4 * j, 4 * j + 4):
                    emit_hT(kc)
            for j in range(1, NQ // QC):
                emit_f(j)

            # ---- main loop over query chunks ----
            # v-projection of chunk qi is emitted in the middle of chunk
            # qi+1's group loop: its wait on the softmax-divide chain
            # then overlaps the next chunk's logits stream instead of
            # head-of-line blocking the PE.
            def emit_v(qj, x0a):
                for oc in range(2):
                    vps = pp.tile([128, QC], F32, tag="pp")
                    _mm(nc, vps, wvT_sb[:, oc, :], x0a, True, True)
                    ot = outp.tile([128, QC], F32)
                    nc.vector.tensor_add(
                        ot, vps, x_sb[:, oc, ts(qj, QC)].bitcast(F32)
                    )
                    nc.sync.dma_start(out=outr[:, oc, ts(qj, QC)], in_=ot)

            pending_v = None
            for qi in range(NQC):
                # row 0: softmax denominator (ones column in hT);
                # rows 1-32: x0 channels.
                x0 = px0.tile([CI + 1, QC], F32)
                for g0 in range(0, NKC, GRP):
                    if g0 == 4 * GRP and pending_v is not None:
                        emit_v(*pending_v)
                        pending_v = None
                    ps = pl.tile([128, GRP, QC], F32, tag="lg")
                    eT = eTp.tile([128, GRP, QC], F32R)
                    for j in range(GRP):
                        kc = g0 + j
                        # row-packed: strip kc%4 holds its own copy of
                        # g/f, so adjacent matmuls execute concurrently
                        # in different PE row bands.
                        s = kc % 4
                        sl = slice(32 * s, 32 * (s + 1))
                        nc.tensor.matmul(
                            ps[:, j, :],
                            lhsT=g_sb[sl, ts(kc, KC)],
                            rhs=f_sb[sl, ts(qi, QC)],
                            start=True, stop=True,
                            tile_position=(32 * s, 0),
                        )
                    nc.scalar.activation(
                        out=eT[:, :, :], in_=ps[:, :, :], func=EXP
                    )
                    for j in range(GRP):
                        kc = g0 + j
                        _mm(nc, x0, hT_sb[:, kc, :], eT[:, j, :],
                            kc == 0, kc == NKC - 1)
                # softmax divide: row 0 of x0 is the denominator
                rcp = smallp.tile([1, QC], F32, tag="rcp")
                nc.vector.reciprocal(rcp, x0[0:1, :])
                rcp_b = smallp.tile([CI + 1, QC], F32, tag="rcpb")
                nc.gpsimd.partition_broadcast(rcp_b, rcp)
                x0a = smallp.tile([CI + 1, QC], F32R, tag="x0a")
                nc.vector.tensor_mul(x0a, x0, rcp_b)
                pending_v = (qi, x0a)
            emit_v(*pending_v)

    nc.compile()
    return nc


def kernel(x, wf, bf, wg, bg, wh, bh, wv, bv, gamma):
    global _cached_nc, LAST_EXEC_NS
    if _cached_nc is None:
        _cached_nc = _build()
    nc = _cached_nc

    x = np.asarray(x, dtype=np.float32)
    wf = np.asarray(wf, dtype=np.float32)
    bf = np.asarray(bf, dtype=np.float32)
    wg = np.asarray(wg, dtype=np.float32)
    bg = np.asarray(bg, dtype=np.float32)
    wh = np.asarray(wh, dtype=np.float32)
    bh = np.asarray(bh, dtype=np.float32)
    wv = np.asarray(wv, dtype=np.float32)
    bv = np.asarray(bv, dtype=np.float32)
    g0 = float(np.asarray(gamma, dtype=np.float32).reshape(-1)[0])

    xf = np.ascontiguousarray(x.reshape(B, C, N))
    # f/g weights replicated 4x along M so f/g land replicated on the
    # four 32-partition strips (enables row-packed logits matmuls).
    wfT = np.ascontiguousarray(np.tile(wf.T, (1, 4)))     # (256, 128)
    wgT = np.ascontiguousarray(np.tile(wg.T, (1, 4)))     # (256, 128)
    whT = np.ascontiguousarray(wh.T)                      # (256, 32)
    wvT = np.empty((CI + 1, C), np.float32)               # aug: bias row 0
    wvT[0, :] = g0 * (bv + wv @ bh)
    wvT[1:, :] = g0 * wv.T
    bf4 = np.ascontiguousarray(np.tile(bf, 4).reshape(128, 1))
    bg4 = np.ascontiguousarray(np.tile(bg, 4).reshape(128, 1))

    in_maps = []
    for core in range(NCORES):
        b, half = divmod(core, 2)
        xb = xf[b]
        if half:
            xb = np.ascontiguousarray(
                np.concatenate([xb[:, NQ:], xb[:, :NQ]], axis=1)
            )
        in_maps.append(
            {"x": xb, "wfT": wfT, "wgT": wgT, "whT": whT, "wvT": wvT,
             "bf": bf4, "bg": bg4}
        )

    res = run_bass_kernel_spmd(
        nc, in_maps, list(range(NCORES)),
        trace=TRACE or bool(os.environ.get("BASS_KERNEL_TRACE")),
    )
    LAST_EXEC_NS = res.exec_time_ns

    out = np.empty((B, C, N), np.float32)
    for core in range(NCORES):
        b, half = divmod(core, 2)
        out[b][:, half * NQ : (half + 1) * NQ] = res.results[core]["out"]
    return out.reshape(B, C, W, H)


# revision 12
# speedup vs baseline: 1.1498x; 1.1498x over previous
"""Trainium2 Bass kernel for the attention layer:

    f = wf@x+bf; g = wg@x+bg; h = wh@x+bh            (1x1 convs, Ci=32)
    attn = softmax(f^T g, axis=-1)                   (per batch, N=4096)
    out = (wv @ (h @ attn^T) + bv) * gamma + x

Sharding: 8 cores = 4 batches x 2 query-halves (2048 queries each).
Each core receives the full (256, 4096) batch slice with its query half
permuted to the front, so the SPMD program uses fixed offsets.

Per-core dataflow (matmuls fp32r, PSUM fp32 accumulate):
  - warm-up: a dense block of dummy matmuls at t=0 so the PE HAM clock
    gate reaches 8/8 before the real work, plus a dummy exp to pull the
    ACT table load forward.
  - f/g are computed replicated onto 4 partition strips (host-replicated
    wf^T/wg^T with M=128), so the K=32 logits matmuls can be row-packed
    with tile_position: consecutive key chunks run concurrently in PE
    row bands, each writing its own PSUM bank.
  - hT (4096, 32) k-major blocks: lhsT=x k-chunk, rhs=wh^T.
  - per 512-query chunk: 32 k-chunk matmuls logitsT = g^T f (k on
    partitions, row-packed) -> ACT exp PSUM->SBUF (1024 wide) -> 32
    k-chunk accumulation rounds, each round two CONCURRENT column-tiled
    matmuls into one PSUM bank: rows 0-31 accumulate the softmax
    denominator (ones stationary), rows 32-63 accumulate x0 = h@attn^T
    (hT stationary). Reciprocal of row 0, GPSIMD partition-broadcast,
    multiply -> x0a; project with wv*gamma; bias (bv+wv@bh folded on
    host) + residual fused in one scalar_tensor_tensor; DMA out.
"""

import os
import numpy as np

import concourse.bass as bass
import concourse.mybir as mybir
import concourse.tile as tile
from concourse import bacc
from concourse.bass import ts
from concourse.bass_utils import run_bass_kernel_spmd

F32 = mybir.dt.float32
F32R = mybir.dt.float32r
EXP = mybir.ActivationFunctionType.Exp
ADD = mybir.AluOpType.add

B, C, W, H = 4, 256, 64, 64
N = W * H            # 4096 keys/queries per batch
CI = 32              # inner channels
NCORES = 8
NQ = N // 2          # queries per core
QC = 512             # query chunk = one fp32 PSUM bank
NQC = NQ // QC       # 4 query chunks per core
KC = 128             # key chunk = partition dim
NKC = N // KC        # 32 key chunks
GRP = 2              # key chunks per ACT exp group (PSUM banks per tile)
NWARM = 8            # dummy fp32 matmuls to warm the PE clock gate

# Trace knob for test harnesses: set kernel.TRACE = True to profile.
TRACE = False
LAST_EXEC_NS = None

_cached_nc = None


def _mm(nc, out, lhsT, rhs, start, stop, tile_position=None):
    nc.tensor.matmul(out, lhsT=lhsT, rhs=rhs, start=start, stop=stop,
                     tile_position=tile_position)


def _build():
    nc = bacc.Bacc(
        "TRN2", target_bir_lowering=False, debug=False, num_devices=NCORES
    )
    x_d = nc.dram_tensor("x", (C, N), F32R, kind="ExternalInput").ap()
    wfT_d = nc.dram_tensor("wfT", (C, 128), F32R, kind="ExternalInput").ap()
    wgT_d = nc.dram_tensor("wgT", (C, 128), F32R, kind="ExternalInput").ap()
    whT_d = nc.dram_tensor("whT", (C, CI), F32R, kind="ExternalInput").ap()
    wvT_d = nc.dram_tensor("wvT", (CI + 1, C), F32R, kind="ExternalInput").ap()
    bf_d = nc.dram_tensor("bf", (128, 1), F32, kind="ExternalInput").ap()
    bg_d = nc.dram_tensor("bg", (128, 1), F32, kind="ExternalInput").ap()
    out_d = nc.dram_tensor("out", (C, NQ), F32, kind="ExternalOutput").ap()

    xr = x_d.rearrange("(cc p) n -> p cc n", p=128)
    outr = out_d.rearrange("(oc p) n -> p oc n", p=128)

    with tile.TileContext(nc) as tc:
        with (
            tc.tile_pool(name="consts", bufs=1) as consts,
            tc.tile_pool(name="data", bufs=1) as data,
            tc.tile_pool(name="eTp", bufs=4) as eTp,
            tc.tile_pool(name="smallp", bufs=2) as smallp,
            tc.tile_pool(name="outp", bufs=3) as outp,
            tc.tile_pool(name="pl", bufs=2, space="PSUM") as pl,
            tc.tile_pool(name="pp", bufs=3, space="PSUM") as pp,
            tc.tile_pool(name="px0", bufs=1, space="PSUM") as px0,
        ):
            # ---- PE + ACT warm-up (overlaps the input DMAs) ----
            scratch = consts.tile([128, QC], F32)
            nc.vector.memset(scratch, 0.0)
            wps = pp.tile([128, QC], F32, tag="pp")
            for i in range(NWARM):
                nc.tensor.matmul(
                    wps, lhsT=scratch[:, 0:128], rhs=scratch,
                    start=True, stop=True, skip_group_check=True,
                )
            scratch2 = consts.tile([1, 8], F32)
            nc.scalar.activation(
                out=scratch2, in_=scratch[0:1, 0:8], func=EXP
            )

            # ---- constants ----
            wfT_sb = consts.tile([128, 2, 128], F32R)
            nc.sync.dma_start(
                out=wfT_sb, in_=wfT_d.rearrange("(cc p) o -> p cc o", p=128)
            )
            wgT_sb = consts.tile([128, 2, 128], F32R)
            nc.sync.dma_start(
                out=wgT_sb, in_=wgT_d.rearrange("(cc p) o -> p cc o", p=128)
            )
            whT_sb = consts.tile([128, 2, CI], F32R)
            nc.sync.dma_start(
                out=whT_sb, in_=whT_d.rearrange("(cc p) o -> p cc o", p=128)
            )
            wvT_sb = consts.tile([CI + 1, 2, 128], F32R)
            nc.sync.dma_start(
                out=wvT_sb, in_=wvT_d.rearrange("p (oc m) -> p oc m", oc=2)
            )
            bf_sb = consts.tile([128, 1], F32)
            nc.sync.dma_start(out=bf_sb, in_=bf_d)
            bg_sb = consts.tile([128, 1], F32)
            nc.sync.dma_start(out=bg_sb, in_=bg_d)
            ones_sb = consts.tile([128, 1], F32)
            nc.vector.memset(ones_sb, 1.0)
            scratchR = consts.tile([128, QC], F32R)
            nc.vector.tensor_copy(scratchR, scratch)

            # ---- x ----
            x_sb = data.tile([128, 2, N], F32R)
            for s in range(4):
                nc.sync.dma_start(
                    out=x_sb[:, :, ts(s, N // 4)], in_=xr[:, :, ts(s, N // 4)]
                )

            # ---- f, g (replicated on 4 strips), hT ----
            f_sb = data.tile([128, NQ], F32R)
            g_sb = data.tile([128, N], F32R)
            hT_sb = data.tile([128, NKC, CI + 1], F32R)
            nc.vector.tensor_copy(
                hT_sb[:, :, 0:1], ones_sb.to_broadcast([128, NKC, 1])
            )

            def emit_f(j):
                ps = pp.tile([128, QC], F32, tag="pp", name=f"psf{j}")
                for cc in range(2):
                    _mm(nc, ps, wfT_sb[:, cc, :],
                        x_sb[:, cc, ts(j, QC)], cc == 0, cc == 1)
                nc.vector.tensor_scalar_add(
                    f_sb[:, ts(j, QC)], ps, bf_sb
                )

            def emit_g(j):
                ps = pp.tile([128, QC], F32, tag="pp", name=f"psg{j}")
                for cc in range(2):
                    _mm(nc, ps, wgT_sb[:, cc, :],
                        x_sb[:, cc, ts(j, QC)], cc == 0, cc == 1)
                nc.vector.tensor_scalar_add(
                    g_sb[:, ts(j, QC)], ps, bg_sb
                )

            def emit_hT(kc):
                ps = pp.tile([128, QC], F32, tag="pp", name=f"psh{kc}")
                for cc in range(2):
                    _mm(nc, ps[:, 0:CI], x_sb[:, cc, ts(kc, KC)],
                        whT_sb[:, cc, :], cc == 0, cc == 1)
                nc.vector.tensor_copy(hT_sb[:, kc, 1 : CI + 1], ps[:, 0:CI])

            # emit order sets scheduler priority: produce what the main
            # loop's first groups need first, so the q-loop starts early
            # and the rest of this phase fills PE gaps under the
            # ACT-bound steady state.
            emit_f(0)
            for j in range(N // QC):
                emit_g(j)
                for kc in range(4 * j, 4 * j + 4):
                    emit_hT(kc)
            for j in range(1, NQ // QC):
                emit_f(j)

            # ---- main loop over query chunks ----
            for qi in range(NQC):
                # row 0: softmax denominator (ones column in hT);
                # rows 1-32: x0 channels.
                x0 = px0.tile([CI + 1, QC], F32)
                for g0 in range(0, NKC, GRP):
                    ps = pl.tile([128, GRP, QC], F32, tag="lg")
                    eT = eTp.tile([128, GRP, QC], F32R)
                    for j in range(GRP):
                        kc = g0 + j
                        # row-packed: strip kc%4 holds its own copy of
                        # g/f, so adjacent matmuls execute concurrently
                        # in different PE row bands.
                        s = kc % 4
                        sl = slice(32 * s, 32 * (s + 1))
                        nc.tensor.matmul(
                            ps[:, j, :],
                            lhsT=g_sb[sl, ts(kc, KC)],
                            rhs=f_sb[sl, ts(qi, QC)],
                            start=True, stop=True,
                            tile_position=(32 * s, 0),
                        )
                    nc.scalar.activation(
                        out=eT[:, :, :], in_=ps[:, :, :], func=EXP
                    )
                    if qi < 2:
                        # dense filler while the exp of this group runs:
                        # keeps the PE array busy enough for the HAM
                        # clock gate to reach/hold 8/8.
                        wb = pp.tile([128, QC], F32, tag="pp")
                        nc.tensor.matmul(
                            wb, lhsT=scratchR[:, 0:128], rhs=scratchR,
                            start=True, stop=True, skip_group_check=True,
                        )
                    for j in range(GRP):
                        kc = g0 + j
                        _mm(nc, x0, hT_sb[:, kc, :], eT[:, j, :],
                            kc == 0, kc == NKC - 1)
                # softmax divide: row 0 of x0 is the denominator
                rcp = smallp.tile([1, QC], F32, tag="rcp")
                nc.vector.reciprocal(rcp, x0[0:1, :])
                rcp_b = smallp.tile([CI + 1, QC], F32, tag="rcpb")
                nc.gpsimd.partition_broadcast(rcp_b, rcp)
                x0a = smallp.tile([CI + 1, QC], F32R, tag="x0a")
                nc.vector.tensor_mul(x0a, x0, rcp_b)
                # project back to C channels; bias + residual fused
                for oc in range(2):
                    vps = pp.tile([128, QC], F32, tag="pp")
                    _mm(nc, vps, wvT_sb[:, oc, :], x0a, True, True)
                    ot = outp.tile([128, QC], F32)
                    nc.vector.tensor_add(
                        ot, vps, x_sb[:, oc, ts(qi, QC)].bitcast(F32)
                    )
                    nc.sync.dma_start(out=outr[:, oc, ts(qi, QC)], in_=ot)
                if qi < NQC - 1:
                    # dense fp32r dummy matmuls: re-warm the PE clock
                    # gate in case a stall re-throttled it this chunk.
                    wb = pp.tile([128, QC], F32, tag="pp")
                    for i in range(6):
                        nc.tensor.matmul(
                            wb, lhsT=scratchR[:, 0:128], rhs=scratchR,
                            start=True, stop=True, skip_group_check=True,
                        )

    nc.compile()
    return nc


def kernel(x, wf, bf, wg, bg, wh, bh, wv, bv, gamma):
    global _cached_nc, LAST_EXEC_NS
    if _cached_nc is None:
        _cached_nc = _build()
    nc = _cached_nc

    x = np.asarray(x, dtype=np.float32)
    wf = np.asarray(wf, dtype=np.float32)
    bf = np.asarray(bf, dtype=np.float32)
    wg = np.asarray(wg, dtype=np.float32)
    bg = np.asarray(bg, dtype=np.float32)
    wh = np.asarray(wh, dtype=np.float32)
    bh = np.asarray(bh, dtype=np.float32)
    wv = np.asarray(wv, dtype=np.float32)
    bv = np.asarray(bv, dtype=np.float32)
    g0 = float(np.asarray(gamma, dtype=np.float32).reshape(-1)[0])

    xf = np.ascontiguousarray(x.reshape(B, C, N))
    # f/g weights replicated 4x along M so f/g land replicated on the
    # four 32-partition strips (enables row-packed logits matmuls).
    wfT = np.ascontiguousarray(np.tile(wf.T, (1, 4)))     # (256, 128)
    wgT = np.ascontiguousarray(np.tile(wg.T, (1, 4)))     # (256, 128)
    whT = np.ascontiguousarray(wh.T)                      # (256, 32)
    wvT = np.empty((CI + 1, C), np.float32)               # aug: bias row 0
    wvT[0, :] = g0 * (bv + wv @ bh)
    wvT[1:, :] = g0 * wv.T
    bf4 = np.ascontiguousarray(np.tile(bf, 4).reshape(128, 1))
    bg4 = np.ascontiguousarray(np.tile(bg, 4).reshape(128, 1))

    in_maps = []
    for core in range(NCORES):
        b, half = divmod(core, 2)
        xb = xf[b]
        if half:
            xb = np.ascontiguousarray(
                np.concatenate([xb[:, NQ:], xb[:, :NQ]], axis=1)
            )
        in_maps.append(
            {"x": xb, "wfT": wfT, "wgT": wgT, "whT": whT, "wvT": wvT,
             "bf": bf4, "bg": bg4}
        )

    res = run_bass_kernel_spmd(
        nc, in_maps, list(range(NCORES)),
        trace=TRACE or bool(os.environ.get("BASS_KERNEL_TRACE")),
    )
    LAST_EXEC_NS = res.exec_time_ns

    out = np.empty((B, C, N), np.float32)
    for core in range(NCORES):
        b, half = divmod(core, 2)
        out[b][:, half * NQ : (half + 1) * NQ] = res.results[core]["out"]
    return out.reshape(B, C, W, H)
